# revision 1
# baseline (speedup 1.0000x reference)
"""Llama GQA attention (B=2, S=2048, HID=2048, H=32, HKV=8, DH=64) on 8 TRN2 cores.

Sharding: tensor-parallel over heads. Core c owns q heads [4c, 4c+4) and kv
head c. One SPMD NEFF per run:
  1. Q/K/V projections in transposed layout (fp32r matmuls at full PE rate),
     interleaved per 512-token block with the attention that consumes them,
  2. RoPE via a signed-permutation matmul + DVE combines,
  3. causal flash attention with scores kept transposed [k, q] so the PV
     matmul needs no on-chip transposes; softmax sums come from a ones-row
     appended to V; no max subtraction (scores are small for this problem);
     diagonal tiles are column-sliced so only the visible staircase is
     computed and a single [128,128] mask handles the mixed subtile,
  4. per-batch AllGather of the normalized context (ctx^T, [256, 2048]),
  5. column-sharded o_proj: each core produces out^T rows [256c, 256c+256).
Host pre-transposes inputs and assembles the 8 output slices.
"""
import sys

sys.path.insert(0, "/opt/trn_rl_repo")

import numpy as np

B, S, HID = 2, 2048, 2048
H, HKV, DH = 32, 8, 64
NC = 8
T = B * S
HPC = H // NC            # q heads per core (4)
CPC = HPC * DH           # ctx dims per core (256)
TB = 512                 # token block
KC = 128                 # k chunk
QBS = S // TB            # 4 q blocks per batch
SB_KC = S // KC          # 16 k chunks per batch
HCH = HID // 128         # 16 hid chunks
SCALE = DH ** -0.5
NEG = -1.0e30


def _build(causal: bool, reps: int = 1, phases: str = "all", bf16: bool = False):
    """phases: 'all' | 'proj' (projections+RoPE only) | 'noop' (skip o_proj+AG).
    bf16: stream hidden/weights/context I/O in bfloat16 (attention stays f32r)."""
    import concourse.mybir as mybir
    import concourse.tile as tile
    from concourse import bacc
    from concourse.masks import make_identity

    F32 = mybir.dt.float32
    F32R = mybir.dt.float32r
    BF16 = mybir.dt.bfloat16
    IOD = BF16 if bf16 else F32R
    EXPF = mybir.ActivationFunctionType.Exp
    ADD = mybir.AluOpType.add
    MUL = mybir.AluOpType.mult

    nc = bacc.Bacc("TRN2", target_bir_lowering=False, debug=False, num_devices=NC)

    hT = nc.dram_tensor("hT", [HID, T], IOD, kind="ExternalInput")
    wqT = nc.dram_tensor("wqT", [HID, CPC], IOD, kind="ExternalInput")
    wkvT = nc.dram_tensor("wkvT", [HID, 2 * DH], IOD, kind="ExternalInput")
    woT = nc.dram_tensor("woT", [H * DH, CPC], IOD, kind="ExternalInput")
    cosT = nc.dram_tensor("cosT", [DH, T], F32, kind="ExternalInput")
    sinT = nc.dram_tensor("sinT", [DH, T], F32, kind="ExternalInput")
    rotp = nc.dram_tensor("rotp", [DH, DH], F32R, kind="ExternalInput")
    if causal:
        maskd = nc.dram_tensor("maskd", [128, 128], F32, kind="ExternalInput")
    else:
        maskg = nc.dram_tensor("maskg", [S, S], F32, kind="ExternalInput")
    outT = nc.dram_tensor("outT", [CPC, T], F32, kind="ExternalOutput")

    with tile.TileContext(nc) as tc:
        with tc.tile_pool(name="const", bufs=1) as cpool, \
             tc.tile_pool(name="big", bufs=1) as big, \
             tc.tile_pool(name="stream", bufs=3) as stream, \
             tc.tile_pool(name="rope", bufs=2) as rope, \
             tc.tile_pool(name="attn", bufs=3) as attn, \
             tc.tile_pool(name="psM", bufs=1, space="PSUM") as psM, \
             tc.tile_pool(name="psS", bufs=3, space="PSUM") as psS, \
             tc.tile_pool(name="psC", bufs=1, space="PSUM") as psC, \
             tc.tile_pool(name="dram", bufs=1, space="DRAM") as dram:

            # ---- persistent SBUF ----
            wq_sb = cpool.tile([128, HCH, CPC], IOD)
            nc.sync.dma_start(wq_sb[:], wqT[:].rearrange("(o p) m -> p o m", p=128))
            wkv_sb = cpool.tile([128, HCH, 2 * DH], IOD)
            nc.sync.dma_start(wkv_sb[:], wkvT[:].rearrange("(o p) m -> p o m", p=128))
            wo_sb = cpool.tile([128, HCH, CPC], IOD)
            nc.sync.dma_start(wo_sb[:], woT[:].rearrange("(o p) m -> p o m", p=128))
            cos_sb = cpool.tile([DH, T], F32)
            nc.sync.dma_start(cos_sb[:], cosT[:])
            sin_sb = cpool.tile([DH, T], F32)
            nc.sync.dma_start(sin_sb[:], sinT[:])
            rot_sb = cpool.tile([DH, DH], F32R)
            nc.sync.dma_start(rot_sb[:], rotp[:])
            if causal:
                mk_sb = cpool.tile([128, 128], F32)
                nc.sync.dma_start(mk_sb[:], maskd[:])
            onesc_f = cpool.tile([128, SB_KC, 1], F32)
            nc.any.memset(onesc_f[:], 1.0)
            ident = cpool.tile([DH, DH], F32)
            make_identity(nc, ident)

            # ---- per-batch big activation buffers ----
            qT_sb = [[big.tile([128, S], F32R, tag=f"qT{b}{hp}", name=f"qT{b}{hp}")
                      for hp in range(2)] for b in range(B)]
            kT_sb = [big.tile([128, S], F32R, tag=f"kT{b}", name=f"kT{b}")
                     for b in range(B)]
            v_sb = [big.tile([128, SB_KC, DH + 1], F32R, tag=f"v{b}", name=f"v{b}")
                    for b in range(B)]
            for b in range(B):
                nc.vector.tensor_copy(v_sb[b][:, :, DH:DH + 1], onesc_f[:])

            ag_in = [[dram.tile([CPC, S], IOD, name=f"agi{b}_{r}")
                      for b in range(B)] for r in range(reps)]
            ag_out = [[dram.tile([H * DH, S], IOD, name=f"ago{b}_{r}",
                                 addr_space="Shared") for b in range(B)]
                      for r in range(reps)]

            def proj_block(b, qb, rep):
                tb = b * QBS + qb
                gs = slice(tb * TB, (tb + 1) * TB)      # global token slice
                ls = slice(qb * TB, (qb + 1) * TB)      # slice within batch
                pq = [psM.tile([128, TB], F32, tag=f"mm{hp}", name=f"pq{hp}_{tb}_{rep}")
                      for hp in range(2)]
                pkv = psM.tile([128, TB], F32, tag="mmkv")
                for cc in range(HCH):
                    h_sb = stream.tile([128, TB], IOD, tag="h")
                    nc.sync.dma_start(h_sb[:], hT[cc * 128:(cc + 1) * 128, gs])
                    for hp in range(2):
                        nc.tensor.matmul(pq[hp][:],
                                         wq_sb[:, cc, hp * 128:(hp + 1) * 128],
                                         h_sb[:], start=(cc == 0),
                                         stop=(cc == HCH - 1))
                    nc.tensor.matmul(pkv[:], wkv_sb[:, cc, :], h_sb[:],
                                     start=(cc == 0), stop=(cc == HCH - 1))
                # RoPE on q heads, one 64-row half at a time
                for h in range(HPC):
                    hp, hh = h // 2, 64 * (h % 2)
                    src = pq[hp][hh:hh + 64, :]
                    qraw = rope.tile([DH, TB], F32R, tag="raw")
                    nc.vector.tensor_copy(qraw[:], src)
                    qcos = rope.tile([DH, TB], F32, tag="cos")
                    nc.vector.tensor_tensor(qcos[:], qraw[:].bitcast(F32),
                                            cos_sb[:, gs], MUL)
                    rps = psS.tile([DH, TB], F32, tag="s")
                    nc.tensor.matmul(rps[:], rot_sb[:], qraw[:], start=True, stop=True)
                    qsin = rope.tile([DH, TB], F32, tag="sin")
                    nc.vector.tensor_tensor(qsin[:], rps[:], sin_sb[:, gs], MUL)
                    if hh == 0:
                        nc.vector.tensor_tensor(qT_sb[b][hp][0:64, ls],
                                                qcos[:], qsin[:], ADD)
                    else:
                        qfin = rope.tile([DH, TB], F32R, tag="fin")
                        nc.vector.tensor_tensor(qfin[:], qcos[:], qsin[:], ADD)
                        nc.sync.dma_start(qT_sb[b][hp][64:128, ls], qfin[:])
                # RoPE on K (kv psum rows 64:128; V in rows 0:64)
                ksrc = pkv[64:128, :]
                kraw = rope.tile([DH, TB], F32R, tag="raw")
                nc.vector.tensor_copy(kraw[:], ksrc)
                kcos = rope.tile([DH, TB], F32, tag="cos")
                nc.vector.tensor_tensor(kcos[:], kraw[:].bitcast(F32),
                                        cos_sb[:, gs], MUL)
                krps = psS.tile([DH, TB], F32, tag="s")
                nc.tensor.matmul(krps[:], rot_sb[:], kraw[:], start=True, stop=True)
                ksin = rope.tile([DH, TB], F32, tag="sin")
                nc.vector.tensor_tensor(ksin[:], krps[:], sin_sb[:, gs], MUL)
                nc.vector.tensor_tensor(kT_sb[b][0:64, ls], kcos[:], ksin[:], ADD)
                # duplicate K^T rows for base-64 matmuls
                nc.sync.dma_start(kT_sb[b][64:128, ls], kT_sb[b][0:64, ls])
                # V: psum rows 0:64 -> transpose into [128, 64] chunks
                vraw = rope.tile([DH, TB], F32, tag="vraw")
                nc.vector.tensor_copy(vraw[:], pkv[0:64, :])
                for i in range(TB // KC):
                    vtp = psS.tile([128, DH], F32, tag="s")
                    nc.tensor.transpose(vtp[:], vraw[:, i * KC:(i + 1) * KC], ident[:])
                    nc.vector.tensor_copy(v_sb[b][:, qb * (TB // KC) + i, 0:DH], vtp[:])

            def attn_block(b, qb, rep):
                for hp in range(2):
                    kcs = list(range(4 * qb + 4)) if causal else list(range(SB_KC))
                    # two heads of the pair run their K=64 score matmuls in
                    # different PE row-groups (bases 0/64) concurrently
                    ctxp = [psC.tile([DH + 1, TB], F32, tag=f"ctx{x}",
                                     name=f"ctx{x}_{rep}_{b}_{qb}_{hp}")
                            for x in range(2)]
                    for i, kc in enumerate(kcs):
                        c0 = 128 * (kc - 4 * qb) if (causal and kc >= 4 * qb) else 0
                        qsl = slice(qb * TB + c0, (qb + 1) * TB)
                        sps = [psS.tile([128, TB], F32, tag="s",
                                        name=f"s{x}_{rep}_{b}_{qb}_{hp}_{kc}")
                               for x in range(2)]
                        for x, hh in enumerate((0, 64)):
                            nc.tensor.matmul(
                                sps[x][:, c0:TB],
                                kT_sb[b][hh:hh + 64, kc * KC:(kc + 1) * KC],
                                qT_sb[b][hp][hh:hh + 64, qsl],
                                start=True, stop=True)
                        for x in range(2):
                            if causal:
                                if kc >= 4 * qb:
                                    nc.vector.tensor_tensor(
                                        sps[x][:, c0:c0 + 128],
                                        sps[x][:, c0:c0 + 128], mk_sb[:], ADD)
                            else:
                                mg = attn.tile([128, TB], F32, tag="mg")
                                nc.sync.dma_start(
                                    mg[:], maskg[kc * KC:(kc + 1) * KC,
                                                 qb * TB:(qb + 1) * TB])
                                nc.vector.tensor_tensor(sps[x][:], sps[x][:],
                                                        mg[:], ADD)
                            p_sb = attn.tile([128, TB], F32R, tag="p")
                            nc.scalar.activation(p_sb[:, c0:TB], sps[x][:, c0:TB],
                                                 EXPF, scale=SCALE)
                            nc.tensor.matmul(ctxp[x][:, c0:TB], v_sb[b][:, kc, :],
                                             p_sb[:, c0:TB], start=(i == 0),
                                             stop=(i == len(kcs) - 1),
                                             skip_group_check=True)
                    for x in range(2):
                        h = 2 * hp + x
                        rc = attn.tile([1, TB], F32R, tag="rc")
                        with nc.allow_low_precision(reason="f32r rounding ~1e-4"):
                            nc.vector.reciprocal(rc[:], ctxp[x][DH:DH + 1, :])
                        rb = attn.tile([DH, TB], F32R, tag="rb")
                        nc.gpsimd.partition_broadcast(rb[:], rc[:])
                        ctxn = attn.tile([DH, TB], IOD, tag="ctxn")
                        nc.vector.tensor_tensor(ctxn[:], ctxp[x][0:DH, :],
                                                rb[:].bitcast(mybir.dt.float32), MUL)
                        nc.sync.dma_start(
                            ag_in[rep][b][h * DH:(h + 1) * DH,
                                          qb * TB:(qb + 1) * TB],
                            ctxn[:])

            for rep in range(reps):
                for b in range(B):
                    for qb in range(QBS):
                        proj_block(b, qb, rep)
                        if phases != "proj":
                            attn_block(b, qb, rep)
                    if phases == "proj":
                        # flush accumulators so the phase is observable
                        for hp in range(2):
                            dbg = stream.tile([128, TB], F32, tag="o")
                            nc.vector.tensor_copy(dbg[:], qT_sb[b][hp][:, 0:TB].bitcast(F32))
                            nc.sync.dma_start(outT[hp * 128:(hp + 1) * 128,
                                                   b * S:b * S + TB], dbg[:])
                        continue
                    if phases == "noop":
                        continue
                    nc.gpsimd.collective_compute(
                        "AllGather", mybir.AluOpType.bypass,
                        replica_groups=[list(range(NC))],
                        ins=[ag_in[rep][b].opt()], outs=[ag_out[rep][b].opt()])

                if phases != "all":
                    continue
                # ---- o_proj: out^T slice [CPC, T] ----
                for b in range(B):
                    for qb in range(QBS):
                        ls = slice(qb * TB, (qb + 1) * TB)
                        gs = slice((b * QBS + qb) * TB, (b * QBS + qb + 1) * TB)
                        po = [psM.tile([128, TB], F32, tag=f"mm{o}",
                                       name=f"po{o}_{b}_{qb}_{rep}")
                              for o in range(2)]
                        for cc in range(HCH):
                            c_sb = stream.tile([128, TB], IOD, tag="c")
                            nc.sync.dma_start(
                                c_sb[:], ag_out[rep][b][cc * 128:(cc + 1) * 128, ls])
                            for o in range(2):
                                nc.tensor.matmul(po[o][:],
                                                 wo_sb[:, cc, o * 128:(o + 1) * 128],
                                                 c_sb[:], start=(cc == 0),
                                                 stop=(cc == HCH - 1))
                        for o in range(2):
                            o_sb = stream.tile([128, TB], F32, tag="o")
                            nc.vector.tensor_copy(o_sb[:], po[o][:])
                            nc.sync.dma_start(outT[o * 128:(o + 1) * 128, gs],
                                              o_sb[:])

    nc.compile()
    return nc


def _host_inputs(hidden_states, cos, sin, attention_mask, Wq, Wk, Wv, Wo, causal,
                 bf16=False):
    hT = np.ascontiguousarray(hidden_states.reshape(T, HID).T)
    cosT = np.ascontiguousarray(cos.reshape(T, DH).T)
    sinT = np.ascontiguousarray(sin.reshape(T, DH).T)
    # rot_half as a signed permutation: rot[d] = -x[d+32] (d<32), +x[d-32] (d>=32)
    p64 = np.zeros((DH, DH), np.float32)
    for m in range(32):
        p64[m + 32, m] = -1.0
        p64[m, m + 32] = 1.0
    WqT = np.ascontiguousarray(Wq.T)      # [HID, H*DH]
    WkT = np.ascontiguousarray(Wk.T)      # [HID, HKV*DH]
    WvT = np.ascontiguousarray(Wv.T)
    WoT = np.ascontiguousarray(Wo.T)      # [H*DH, HID]

    if bf16:
        import ml_dtypes
        bf = ml_dtypes.bfloat16
        hT = hT.astype(bf)
        WqT, WkT, WvT, WoT = (w.astype(bf) for w in (WqT, WkT, WvT, WoT))
    ins = []
    for c in range(NC):
        d = {
            "hT": hT,
            "wqT": np.ascontiguousarray(WqT[:, c * CPC:(c + 1) * CPC]),
            "wkvT": np.ascontiguousarray(
                np.concatenate([WvT[:, c * DH:(c + 1) * DH],
                                WkT[:, c * DH:(c + 1) * DH]], axis=1)),
            "woT": np.ascontiguousarray(WoT[:, c * CPC:(c + 1) * CPC]),
            "cosT": cosT, "sinT": sinT, "rotp": p64,
        }
        if causal:
            i = np.arange(128, dtype=np.float32)[:, None]
            cc = np.arange(128, dtype=np.float32)[None, :]
            d["maskd"] = np.where(cc < i, NEG, 0.0).astype(np.float32)
        else:
            m = attention_mask[0, 0].astype(np.float32)
            d["maskg"] = np.ascontiguousarray(m.T) * np.float32(1.0 / SCALE)
        ins.append(d)
    return ins


def _is_causal(attention_mask):
    if attention_mask.shape != (1, 1, S, S):
        return False
    m = attention_mask[0, 0]
    neg = np.finfo(np.float32).min
    tril = np.tril(np.ones((S, S), dtype=bool))
    expect = np.where(tril, np.float32(0.0), np.float32(neg))
    return np.array_equal(m, expect)


_CACHE = {}


BF16_IO = False


def _get_nc(causal, reps=1, phases="all", bf16=None):
    if bf16 is None:
        bf16 = BF16_IO
    key = (causal, reps, phases, bf16)
    if key not in _CACHE:
        _CACHE[key] = _build(causal, reps, phases, bf16)
    return _CACHE[key]


def kernel(**inputs) -> np.ndarray:
    from concourse.bass_utils import run_bass_kernel_spmd

    hidden_states = np.asarray(inputs["hidden_states"], np.float32)
    cos = np.asarray(inputs["cos"], np.float32)
    sin = np.asarray(inputs["sin"], np.float32)
    attention_mask = np.asarray(inputs["attention_mask"], np.float32)
    Wq = np.asarray(inputs["Wq"], np.float32)
    Wk = np.asarray(inputs["Wk"], np.float32)
    Wv = np.asarray(inputs["Wv"], np.float32)
    Wo = np.asarray(inputs["Wo"], np.float32)

    causal = _is_causal(attention_mask)
    nc = _get_nc(causal)
    ins = _host_inputs(hidden_states, cos, sin, attention_mask,
                       Wq, Wk, Wv, Wo, causal, bf16=BF16_IO)
    res = run_bass_kernel_spmd(nc, ins, core_ids=list(range(NC)))
    outT = np.concatenate([res.results[c]["outT"] for c in range(NC)], axis=0)
    return np.ascontiguousarray(outT.T).reshape(B, S, HID)



# revision 22
# speedup vs baseline: 1.2364x; 1.2364x over previous
"""Llama GQA attention (B=2, S=2048, HID=2048, H=32, HKV=8, DH=64) on 8 TRN2 cores.

Sharding: tensor-parallel over heads. Core c owns q heads [4c, 4c+4) and kv
head c. One SPMD NEFF per run. All matmul operands stream in bfloat16
(accumulation in fp32 PSUM); exp softmax without max-subtraction.

Fully software-pipelined causal path: the attention inner loop is the
envelope; every other piece of work is a "feed unit" interleaved into its
PE/exp gaps:
  - score matmuls for chunk k+1 issue while exp(k) runs on the Act engine;
    PV matmuls lag one chunk (so PE never waits on exp latency),
  - causal masking is a 0/1 multiply on the exp output (gpsimd engine, off
    the DVE/Act critical path; masked entries never reach PV or the softmax
    denominator, which comes from a ones-row appended to V),
  - Q-projection of block g+2, KV-projection + RoPE of block g+1, and
    o_proj of block g-2 are all fed between attention matmuls,
  - per-block context AllGather ([256,512] -> [2048,512] shared) issues
    right after each block's attention; o_proj consumes it 2 blocks later.
Hidden/context blocks stage through double-buffered SBUF with one batched
DMA per block. PSUM (8 banks): 3 score + 2 q-proj + 2 ctx + 1 kv/o_proj.
Host pre-transposes inputs and assembles the 8 output slices.
"""
import sys

sys.path.insert(0, "/opt/trn_rl_repo")

import numpy as np

B, S, HID = 2, 2048, 2048
H, HKV, DH = 32, 8, 64
NC = 8
T = B * S
HPC = H // NC            # q heads per core (4)
CPC = HPC * DH           # ctx dims per core (256)
TB = 512                 # token block
KC = 128                 # k chunk
QBS = S // TB            # 4 q blocks per batch
NTB = B * QBS            # 8 token blocks total
SB_KC = S // KC          # 16 k chunks per batch
HCH = HID // 128         # 16 hid chunks
SCALE = DH ** -0.5
NEG = -1.0e30


def _build(causal: bool, reps: int = 1, phases: str = "all", bf16: bool = True):
    import concourse.mybir as mybir
    import concourse.tile as tile
    from concourse import bacc
    from concourse.masks import make_identity

    F32 = mybir.dt.float32
    F32R = mybir.dt.float32r
    BF16 = mybir.dt.bfloat16
    IOD = BF16 if bf16 else F32R
    QKD = BF16 if bf16 else F32R
    EXPF = (mybir.ActivationFunctionType.Copy if phases == "noexp"
            else mybir.ActivationFunctionType.Exp)
    ADD = mybir.AluOpType.add
    MUL = mybir.AluOpType.mult

    nc = bacc.Bacc("TRN2", target_bir_lowering=False, debug=False, num_devices=NC)

    hT = nc.dram_tensor("hT", [HID, T], IOD, kind="ExternalInput")
    wqT = nc.dram_tensor("wqT", [HID, CPC], IOD, kind="ExternalInput")
    wkvT = nc.dram_tensor("wkvT", [HID, 2 * DH], IOD, kind="ExternalInput")
    woT = nc.dram_tensor("woT", [H * DH, CPC], IOD, kind="ExternalInput")
    cosT = nc.dram_tensor("cosT", [DH, T], F32, kind="ExternalInput")
    sinT = nc.dram_tensor("sinT", [DH, T], F32, kind="ExternalInput")
    rotp = nc.dram_tensor("rotp", [2 * DH, 2 * DH], QKD, kind="ExternalInput")
    if causal:
        maskd = nc.dram_tensor("maskd", [128, 128], QKD, kind="ExternalInput")
    else:
        maskg = nc.dram_tensor("maskg", [S, S], F32, kind="ExternalInput")
    outT = nc.dram_tensor("outT", [CPC, T], F32, kind="ExternalOutput")

    G = reps * NTB  # global block count

    with tile.TileContext(nc) as tc:
        with tc.tile_pool(name="const", bufs=1) as cpool, \
             tc.tile_pool(name="big", bufs=1) as big, \
             tc.tile_pool(name="pb", bufs=6) as pb, \
             tc.tile_pool(name="rope", bufs=2) as rope, \
             tc.tile_pool(name="attn", bufs=4) as attn, \
             tc.tile_pool(name="outp", bufs=2) as outp, \
             tc.tile_pool(name="psS", bufs=3, space="PSUM") as psS, \
             tc.tile_pool(name="psM", bufs=1, space="PSUM") as psM, \
             tc.tile_pool(name="psC", bufs=1, space="PSUM") as psC, \
             tc.tile_pool(name="psP", bufs=1, space="PSUM") as psP, \
             tc.tile_pool(name="dram", bufs=1, space="DRAM") as dram:

            # ---- persistent SBUF ----
            wq_sb = cpool.tile([128, HCH, CPC], IOD)
            nc.sync.dma_start(wq_sb[:], wqT[:].rearrange("(o p) m -> p o m", p=128))
            wkv_sb = cpool.tile([128, HCH, 2 * DH], IOD)
            nc.sync.dma_start(wkv_sb[:], wkvT[:].rearrange("(o p) m -> p o m", p=128))
            wo_sb = cpool.tile([128, HCH, CPC], IOD)
            nc.sync.dma_start(wo_sb[:], woT[:].rearrange("(o p) m -> p o m", p=128))
            cos_sb = cpool.tile([2 * DH, T], F32)
            nc.sync.dma_start(cos_sb[0:DH, :], cosT[:])
            nc.sync.dma_start(cos_sb[DH:2 * DH, :], cosT[:])
            sin_sb = cpool.tile([2 * DH, T], F32)
            nc.sync.dma_start(sin_sb[0:DH, :], sinT[:])
            nc.sync.dma_start(sin_sb[DH:2 * DH, :], sinT[:])
            rot_sb = cpool.tile([2 * DH, 2 * DH], QKD)
            nc.sync.dma_start(rot_sb[:], rotp[:])
            if causal:
                mk_sb = cpool.tile([128, 128], QKD)
                nc.sync.dma_start(mk_sb[:], maskd[:])
            onesc_f = cpool.tile([128, SB_KC, 1], F32)
            nc.any.memset(onesc_f[:], 1.0)
            ident = cpool.tile([DH, DH], F32)
            make_identity(nc, ident)

            # ---- per-batch activation buffers ----
            qT_sb = [[big.tile([128, S], QKD, tag=f"qT{b}{hp}", name=f"qT{b}{hp}")
                      for hp in range(2)] for b in range(B)]
            kT_sb = [big.tile([128, S], QKD, tag=f"kT{b}", name=f"kT{b}")
                     for b in range(B)]
            v_sb = [big.tile([128, SB_KC, DH + 1], QKD, tag=f"v{b}", name=f"v{b}")
                    for b in range(B)]
            for b in range(B):
                nc.vector.tensor_copy(v_sb[b][:, :, DH:DH + 1], onesc_f[:])
            # double-buffered hidden / context staging (one block each)
            hbuf = [big.tile([128, HCH, TB], IOD, tag=f"hb{i}", name=f"hb{i}")
                    for i in range(2)]
            cbuf = [big.tile([128, HCH, TB], IOD, tag=f"cb{i}", name=f"cb{i}")
                    for i in range(2)]

            ag_in = [dram.tile([CPC, TB], IOD, name=f"agi{g}") for g in range(G)]
            ag_out = [dram.tile([H * DH, TB], IOD, name=f"ago{g}",
                                addr_space="Shared") for g in range(G)]

            def as_f(x):
                return x if bf16 else x.bitcast(F32)

            def gsl(g):
                tb = g % NTB
                return slice(tb * TB, (tb + 1) * TB)

            # ---------------- feed-unit builders ----------------
            def proj_q_units(g, st):
                hb = hbuf[g % 2]
                gs = gsl(g)

                def dma(cc):
                    def run():
                        nc.sync.dma_start(hb[:, cc, :],
                                          hT[cc * 128:(cc + 1) * 128, gs])
                    return run

                def mm(cc):
                    def run():
                        if cc == 0:
                            st["pq"] = [
                                psM.tile([128, TB], F32, tag=f"mm{hp}",
                                         name=f"pq{hp}_{g}") for hp in range(2)]
                        for hp in range(2):
                            nc.tensor.matmul(st["pq"][hp][:],
                                             wq_sb[:, cc, hp * 128:(hp + 1) * 128],
                                             hb[:, cc, :], start=(cc == 0),
                                             stop=(cc == HCH - 1),
                                             skip_group_check=True)
                    return run

                units = [[dma(0), dma(1)]]
                for cc in range(HCH):
                    u = []
                    if cc + 2 < HCH:
                        u.append(dma(cc + 2))
                    u.append(mm(cc))
                    units.append(u)
                return units

            def kv_units(g, st):
                hb = hbuf[g % 2]

                def mm(cc):
                    def run():
                        if cc == 0:
                            st["pkv"] = psP.tile([128, TB], F32, tag="po",
                                                 name=f"pkv_{g}")
                        for c in (cc, cc + 1):
                            nc.tensor.matmul(st["pkv"][:], wkv_sb[:, c, :],
                                             hb[:, c, :], start=(c == 0),
                                             stop=(c == HCH - 1),
                                             skip_group_check=True)
                    return run

                return [[mm(cc)] for cc in range(0, HCH, 2)]

            def rope_q_units(g, st):
                tb = g % NTB
                b, qb = tb // QBS, tb % QBS
                gs, ls = gsl(g), slice(qb * TB, (qb + 1) * TB)

                def pair_a(hp):
                    def run():
                        st[f"qraw{hp}"] = rope.tile([128, TB], QKD, tag="raw",
                                                    name=f"qraw{hp}_{g}")
                        nc.vector.tensor_copy(st[f"qraw{hp}"][:], st["pq"][hp][:])
                        st[f"qcos{hp}"] = rope.tile([128, TB], F32, tag="cos",
                                                    name=f"qcos{hp}_{g}")
                        nc.vector.tensor_tensor(st[f"qcos{hp}"][:],
                                                as_f(st[f"qraw{hp}"][:]),
                                                cos_sb[:, gs], MUL)
                    return run

                def pair_b(hp):
                    def run():
                        rps = psS.tile([128, TB], F32, tag="s")
                        nc.tensor.matmul(rps[:], rot_sb[:], st[f"qraw{hp}"][:],
                                         start=True, stop=True)
                        qsin = rope.tile([128, TB], F32, tag="sin")
                        nc.vector.tensor_tensor(qsin[:], rps[:], sin_sb[:, gs], MUL)
                        nc.vector.tensor_tensor(qT_sb[b][hp][:, ls],
                                                st[f"qcos{hp}"][:], qsin[:], ADD)
                    return run

                units = []
                for hp in range(2):
                    units += [[pair_a(hp)], [pair_b(hp)]]
                return units

            def rope_kv_units(g, st):
                tb = g % NTB
                b, qb = tb // QBS, tb % QBS
                gs, ls = gsl(g), slice(qb * TB, (qb + 1) * TB)

                def kpart_a():
                    st["kraw"] = rope.tile([DH, TB], QKD, tag="raw", name=f"kraw_{g}")
                    nc.vector.tensor_copy(st["kraw"][:], st["pkv"][64:128, :])
                    st["kcos"] = rope.tile([DH, TB], F32, tag="cos", name=f"kcos_{g}")
                    nc.vector.tensor_tensor(st["kcos"][:], as_f(st["kraw"][:]),
                                            cos_sb[0:DH, gs], MUL)
                    st["vraw"] = rope.tile([DH, TB], F32, tag="vraw", name=f"vraw_{g}")
                    nc.vector.tensor_copy(st["vraw"][:], st["pkv"][0:64, :])

                def kpart_b():
                    krps = psS.tile([DH, TB], F32, tag="s")
                    nc.tensor.matmul(krps[:], rot_sb[0:DH, 0:DH], st["kraw"][:],
                                     start=True, stop=True)
                    ksin = rope.tile([DH, TB], F32, tag="sin")
                    nc.vector.tensor_tensor(ksin[:], krps[:], sin_sb[0:DH, gs], MUL)
                    nc.vector.tensor_tensor(kT_sb[b][0:64, ls], st["kcos"][:],
                                            ksin[:], ADD)
                    nc.sync.dma_start(kT_sb[b][64:128, ls], kT_sb[b][0:64, ls])

                def vpart(i):
                    def run():
                        vtp = psS.tile([128, DH], F32, tag="s")
                        nc.tensor.transpose(vtp[:],
                                            st["vraw"][:, i * KC:(i + 1) * KC],
                                            ident[:])
                        nc.vector.tensor_copy(
                            v_sb[b][:, qb * (TB // KC) + i, 0:DH], vtp[:])
                    return run

                return [[kpart_a], [kpart_b]] + [[vpart(i)] for i in range(TB // KC)]

            def oproj_units(g, st):
                cb = cbuf[g % 2]
                gs = gsl(g)

                def dma():
                    nc.sync.dma_start(
                        cb[:],
                        ag_out[g][:].rearrange("(o p) m -> p o m", p=128))

                def mm(o, cc):
                    def run():
                        if cc == 0:
                            st[f"po{o}"] = psP.tile([128, TB], F32, tag="po",
                                                    name=f"po{o}_{g}")
                        nc.tensor.matmul(st[f"po{o}"][:],
                                         wo_sb[:, cc, o * 128:(o + 1) * 128],
                                         cb[:, cc, :], start=(cc == 0),
                                         stop=(cc == HCH - 1),
                                         skip_group_check=True)
                    return run

                def out(o):
                    def run():
                        o_sb = outp.tile([128, TB], F32, tag="o")
                        nc.vector.tensor_copy(o_sb[:], st[f"po{o}"][:])
                        nc.sync.dma_start(outT[o * 128:(o + 1) * 128, gs], o_sb[:])
                    return run

                units = [[dma]] + [[mm(0, cc)] for cc in range(HCH)]
                units.append([out(0)])
                units += [[mm(1, cc)] for cc in range(HCH)]
                units.append([out(1)])
                return units

            # ---------------- pipeline driver ----------------
            class Feeder:
                def __init__(self):
                    self.units = []

                def push(self, units):
                    self.units.extend(units)

                def feed(self, n=1):
                    for _ in range(n):
                        if not self.units:
                            return
                        for op in self.units.pop(0):
                            op()

                def drain(self):
                    self.feed(len(self.units))

            def attn_block(g, fd):
                tb = g % NTB
                b, qb = tb // QBS, tb % QBS
                for hp in range(2):
                    kcs = list(range(4 * qb + 4))
                    ctxp = [psC.tile([DH + 1, TB], F32, tag=f"ctx{x}",
                                     name=f"ctx{x}_{g}_{hp}")
                            for x in range(2)]
                    prev = None

                    def pv(st_):
                        i, kc, c0, p = st_
                        for x in range(2):
                            nc.tensor.matmul(ctxp[x][:, c0:TB], v_sb[b][:, kc, :],
                                             p[x][:, c0:TB], start=(i == 0),
                                             stop=(i == len(kcs) - 1),
                                             skip_group_check=True)

                    for i, kc in enumerate(kcs):
                        c0 = 128 * (kc - 4 * qb) if kc >= 4 * qb else 0
                        qsl = slice(qb * TB + c0, (qb + 1) * TB)
                        sps = [psS.tile([128, TB], F32, tag="s",
                                        name=f"s{x}_{g}_{hp}_{kc}")
                               for x in range(2)]
                        for x, hh in enumerate((0, 64)):
                            nc.tensor.matmul(
                                sps[x][:, c0:TB],
                                kT_sb[b][hh:hh + 64, kc * KC:(kc + 1) * KC],
                                qT_sb[b][hp][hh:hh + 64, qsl],
                                start=True, stop=True)
                            fd.feed(1)
                        p = [pb.tile([128, TB], QKD, tag="p",
                                     name=f"p{x}_{g}_{hp}_{kc}")
                             for x in range(2)]
                        for x in range(2):
                            nc.scalar.activation(p[x][:, c0:TB], sps[x][:, c0:TB],
                                                 EXPF, scale=SCALE)
                            if kc >= 4 * qb:
                                # zero future positions of the diagonal subtile
                                nc.vector.tensor_tensor(
                                    p[x][:, c0:c0 + 128],
                                    p[x][:, c0:c0 + 128], mk_sb[:], MUL)
                        fd.feed(1)
                        if prev is not None:
                            pv(prev)
                            fd.feed(2)
                        prev = (i, kc, c0, p)
                    pv(prev)
                    # normalize + stage for AllGather
                    for x in range(2):
                        h = 2 * hp + x
                        rc = attn.tile([1, TB], F32R, tag="rc")
                        with nc.allow_low_precision(reason="f32r rounding ~1e-4"):
                            nc.vector.reciprocal(rc[:], ctxp[x][DH:DH + 1, :])
                        rb = attn.tile([DH, TB], F32R, tag="rb")
                        nc.gpsimd.partition_broadcast(rb[:], rc[:])
                        ctxn = attn.tile([DH, TB], IOD, tag="ctxn")
                        nc.vector.tensor_tensor(ctxn[:], ctxp[x][0:DH, :],
                                                rb[:].bitcast(F32), MUL)
                        nc.sync.dma_start(ag_in[g][h * DH:(h + 1) * DH, :], ctxn[:])
                    fd.feed(1)

            def ag_block(g):
                nc.gpsimd.collective_compute(
                    "AllGather", mybir.AluOpType.bypass,
                    replica_groups=[list(range(NC))],
                    ins=[ag_in[g].opt()], outs=[ag_out[g].opt()])

            def run_units(units):
                for u in units:
                    for op in u:
                        op()

            states = [dict() for _ in range(G + 2)]

            def full_prep(g):
                run_units(proj_q_units(g, states[g]))
                run_units(kv_units(g, states[g]))
                run_units(rope_q_units(g, states[g]))
                run_units(rope_kv_units(g, states[g]))

            if causal and phases == "proj":
                for g in range(G):
                    full_prep(g)
                    b = (g % NTB) // QBS
                    if g % NTB == NTB - 1:
                        for hp in range(2):
                            dbg = outp.tile([128, TB], F32, tag="o")
                            nc.vector.tensor_copy(
                                dbg[:], as_f(qT_sb[b][hp][:, 0:TB]))
                            nc.sync.dma_start(
                                outT[hp * 128:(hp + 1) * 128, b * S:b * S + TB],
                                dbg[:])
            elif causal:
                fd = Feeder()
                full_prep(0)
                for g in range(G):
                    pu = (proj_q_units(g + 1, states[g + 1])
                          if g + 1 < G else [])
                    ou = (oproj_units(g - 2, states[g - 2])
                          if (phases in ("all", "noexp") and g >= 2) else [])
                    if phases in ("all", "noexp") and g == G - 1:
                        ou = ou + oproj_units(G - 2, states[G - 2])
                    mixed = []
                    for i in range(max(len(pu), len(ou))):
                        if i < len(ou):
                            mixed.append(ou[i])
                        if i < len(pu):
                            mixed.append(pu[i])
                    fd.push(mixed)
                    if g + 1 < G:
                        fd.push(kv_units(g + 1, states[g + 1]))
                    attn_block(g, fd)
                    if phases in ("all", "noexp"):
                        ag_block(g)
                    fd.drain()
                    if g + 1 < G:
                        run_units(rope_q_units(g + 1, states[g + 1]))
                        run_units(rope_kv_units(g + 1, states[g + 1]))
                if phases in ("all", "noexp"):
                    run_units(oproj_units(G - 1, states[G - 1]))
            else:
                # non-causal fallback: simple serial structure
                for g in range(G):
                    full_prep(g)
                    tb = g % NTB
                    b, qb = tb // QBS, tb % QBS
                    for hp in range(2):
                        kcs = list(range(SB_KC))
                        ctxp = [psC.tile([DH + 1, TB], F32, tag=f"ctx{x}",
                                         name=f"nc_ctx{x}_{g}_{hp}")
                                for x in range(2)]
                        for i, kc in enumerate(kcs):
                            sps = [psS.tile([128, TB], F32, tag="s",
                                            name=f"nc_s{x}_{g}_{hp}_{kc}")
                                   for x in range(2)]
                            for x, hh in enumerate((0, 64)):
                                nc.tensor.matmul(
                                    sps[x][:],
                                    kT_sb[b][hh:hh + 64, kc * KC:(kc + 1) * KC],
                                    qT_sb[b][hp][hh:hh + 64,
                                                 qb * TB:(qb + 1) * TB],
                                    start=True, stop=True)
                            for x in range(2):
                                mg = attn.tile([128, TB], F32, tag="mg")
                                nc.sync.dma_start(
                                    mg[:], maskg[kc * KC:(kc + 1) * KC,
                                                 qb * TB:(qb + 1) * TB])
                                nc.vector.tensor_tensor(sps[x][:], sps[x][:],
                                                        mg[:], ADD)
                                p_sb = attn.tile([128, TB], QKD, tag="p")
                                nc.scalar.activation(p_sb[:], sps[x][:],
                                                     EXPF, scale=SCALE)
                                nc.tensor.matmul(ctxp[x][:], v_sb[b][:, kc, :],
                                                 p_sb[:], start=(i == 0),
                                                 stop=(i == len(kcs) - 1),
                                                 skip_group_check=True)
                        for x in range(2):
                            h = 2 * hp + x
                            rc = attn.tile([1, TB], F32R, tag="rc")
                            with nc.allow_low_precision(reason="f32r rounding"):
                                nc.vector.reciprocal(rc[:], ctxp[x][DH:DH + 1, :])
                            rb = attn.tile([DH, TB], F32R, tag="rb")
                            nc.gpsimd.partition_broadcast(rb[:], rc[:])
                            ctxn = attn.tile([DH, TB], IOD, tag="ctxn")
                            nc.vector.tensor_tensor(ctxn[:], ctxp[x][0:DH, :],
                                                    rb[:].bitcast(F32), MUL)
                            nc.sync.dma_start(ag_in[g][h * DH:(h + 1) * DH, :],
                                              ctxn[:])
                    if phases != "noop":
                        ag_block(g)
                        run_units(oproj_units(g, states[g]))

    nc.compile()
    return nc


def _host_inputs(hidden_states, cos, sin, attention_mask, Wq, Wk, Wv, Wo, causal,
                 bf16=True):
    hT = np.ascontiguousarray(hidden_states.reshape(T, HID).T)
    cosT = np.ascontiguousarray(cos.reshape(T, DH).T)
    sinT = np.ascontiguousarray(sin.reshape(T, DH).T)
    # rot_half as a signed permutation: rot[d] = -x[d+32] (d<32), +x[d-32] (d>=32)
    p64 = np.zeros((DH, DH), np.float32)
    for m in range(32):
        p64[m + 32, m] = -1.0
        p64[m, m + 32] = 1.0
    p64 = np.block([[p64, np.zeros((DH, DH), np.float32)],
                    [np.zeros((DH, DH), np.float32), p64]]).astype(np.float32)
    WqT = np.ascontiguousarray(Wq.T)      # [HID, H*DH]
    WkT = np.ascontiguousarray(Wk.T)      # [HID, HKV*DH]
    WvT = np.ascontiguousarray(Wv.T)
    WoT = np.ascontiguousarray(Wo.T)      # [H*DH, HID]

    if bf16:
        import ml_dtypes
        bf = ml_dtypes.bfloat16
        hT = hT.astype(bf)
        WqT, WkT, WvT, WoT = (w.astype(bf) for w in (WqT, WkT, WvT, WoT))
        p64 = p64.astype(bf)
    ins = []
    for c in range(NC):
        d = {
            "hT": hT,
            "wqT": np.ascontiguousarray(WqT[:, c * CPC:(c + 1) * CPC]),
            "wkvT": np.ascontiguousarray(
                np.concatenate([WvT[:, c * DH:(c + 1) * DH],
                                WkT[:, c * DH:(c + 1) * DH]], axis=1)),
            "woT": np.ascontiguousarray(WoT[:, c * CPC:(c + 1) * CPC]),
            "cosT": cosT, "sinT": sinT, "rotp": p64,
        }
        if causal:
            i = np.arange(128, dtype=np.float32)[:, None]
            cc = np.arange(128, dtype=np.float32)[None, :]
            mk = np.where(cc < i, 0.0, 1.0).astype(np.float32)
            if bf16:
                import ml_dtypes
                mk = mk.astype(ml_dtypes.bfloat16)
            d["maskd"] = mk
        else:
            m = attention_mask[0, 0].astype(np.float32)
            d["maskg"] = np.ascontiguousarray(m.T) * np.float32(1.0 / SCALE)
        ins.append(d)
    return ins


def _is_causal(attention_mask):
    if attention_mask.shape != (1, 1, S, S):
        return False
    m = attention_mask[0, 0]
    neg = np.finfo(np.float32).min
    tril = np.tril(np.ones((S, S), dtype=bool))
    expect = np.where(tril, np.float32(0.0), np.float32(neg))
    return np.array_equal(m, expect)


_CACHE = {}


BF16_IO = True


def _get_nc(causal, reps=1, phases="all", bf16=None):
    if bf16 is None:
        bf16 = BF16_IO
    key = (causal, reps, phases, bf16)
    if key not in _CACHE:
        _CACHE[key] = _build(causal, reps, phases, bf16)
    return _CACHE[key]


def kernel(**inputs) -> np.ndarray:
    from concourse.bass_utils import run_bass_kernel_spmd

    hidden_states = np.asarray(inputs["hidden_states"], np.float32)
    cos = np.asarray(inputs["cos"], np.float32)
    sin = np.asarray(inputs["sin"], np.float32)
    attention_mask = np.asarray(inputs["attention_mask"], np.float32)
    Wq = np.asarray(inputs["Wq"], np.float32)
    Wk = np.asarray(inputs["Wk"], np.float32)
    Wv = np.asarray(inputs["Wv"], np.float32)
    Wo = np.asarray(inputs["Wo"], np.float32)

    causal = _is_causal(attention_mask)
    nc = _get_nc(causal)
    ins = _host_inputs(hidden_states, cos, sin, attention_mask,
                       Wq, Wk, Wv, Wo, causal, bf16=BF16_IO)
    res = run_bass_kernel_spmd(nc, ins, core_ids=list(range(NC)))
    outT = np.concatenate([res.results[c]["outT"] for c in range(NC)], axis=0)
    return np.ascontiguousarray(outT.T).reshape(B, S, HID)
